# revision 23
# baseline (speedup 1.0000x reference)
"""Trainium2 Bass kernel for nn_CrossAttention (B=4, N=2048, D=1024, 16 heads x 64).

Sharding: 8 cores = 4 batches x 2 head-groups (8 heads each).
Each core computes, for its (batch b, head-group g):
    q = x[b] @ Wq[g].T, k/v = ctx[b] @ Wk/Wv[g].T   (E = 512 inner dims)
    per head: ST[j,i] = (k_h q_h^T), P = exp(ST * scale) (no max-sub; logits are small)
    OT_aug = [v_h | 1]^T-accumulated:  matmul(lhsT=v_aug[j,65], rhs=PT[j,i]) -> [65, i]
      rows 0..63 = unnormalized O^T, row 64 = softmax denominator L
    normalize via reciprocal + ones-broadcast matmul
    out_partial = O @ Wo[:, g].T  -> host sums the two partials per batch (+bo)

All activations/weights are fed to the device pre-transposed (layout choice of
the host-side sharding) so every matmul contracts along the partition dim.
"""

import os
import numpy as np

import concourse.bacc as bacc
import concourse.mybir as mybir
import concourse.tile as tile
from concourse.bass_utils import run_bass_kernel_spmd

f32 = mybir.dt.float32
bf16 = mybir.dt.bfloat16

# Problem constants (hardcoded per contract)
B, NQ, NK = 4, 2048, 2048
D = 1024          # model dim
H, DH = 16, 64    # heads, head dim
G = 2             # head-groups (cores per batch)
HC = H // G       # heads per core = 8
E = HC * DH       # per-core inner dim = 512
SCALE = DH ** -0.5
P = 128
DO = D // P       # 8 contraction tiles for projections
EO = E // P       # 4
NJT = NK // P     # 16 j-tiles
NIB = 4           # i-blocks
IBW = NQ // NIB   # 512
NQUAD = 8         # j-quads per (head-pair, i-block): each quad = 2 j-tiles x 2 heads

# Matmul compute dtype: float32r uses the fast PE path (1 cyc/row at N>=256)
# with slightly reduced multiply precision; float32 is exact but 4 cyc/row.
MM_DTYPE = os.environ.get("CROSSATT_MM_DTYPE", "float32r")
# dtype for tensors that feed the TensorEngine (must be produced as this dtype)
MDT = mybir.dt.float32r if MM_DTYPE == "float32r" else f32


def build_nc():
    nc = bacc.Bacc(None)
    xT = nc.declare_dram_parameter("xT", [D, NQ], MDT, isOutput=False)
    cT = nc.declare_dram_parameter("cT", [D, NK], MDT, isOutput=False)
    wqT = nc.declare_dram_parameter("wqT", [D, E], MDT, isOutput=False)
    wkT = nc.declare_dram_parameter("wkT", [D, E], MDT, isOutput=False)
    wvT = nc.declare_dram_parameter("wvT", [D, E], MDT, isOutput=False)
    woT = nc.declare_dram_parameter("woT", [E, D], MDT, isOutput=False)
    outp = nc.declare_dram_parameter("outp", [NQ, D], f32, isOutput=True)

    xT_r = xT.ap().rearrange("(do p) i -> p do i", p=P)
    cT_r = cT.ap().rearrange("(do p) j -> p do j", p=P)
    wqT_r = wqT.ap().rearrange("(do p) e -> p do e", p=P)
    wkT_r = wkT.ap().rearrange("(do p) e -> p do e", p=P)
    wvT_r = wvT.ap().rearrange("(do p) e -> p do e", p=P)
    woT_r = woT.ap().rearrange("(eo p) o -> p eo o", p=P)
    out_ap = outp.ap()

    with tile.TileContext(nc) as tc:
        with (
            tc.tile_pool(name="persist", bufs=1) as persist,
            tc.tile_pool(name="weights", bufs=3) as weights,
            tc.tile_pool(name="stream", bufs=2) as stream,
            tc.tile_pool(name="qpool", bufs=2) as qpool,
            tc.tile_pool(name="opool", bufs=2) as opool,
            tc.tile_pool(name="ptpool", bufs=3) as ptpool,
            tc.tile_pool(name="lrows", bufs=2) as lrowsp,
            tc.tile_pool(name="rcrows", bufs=1) as rcrowsp,
            tc.tile_pool(name="stg", bufs=2) as stg,
            tc.tile_pool(name="ones", bufs=1) as onesp,
            tc.tile_pool(name="ps_st", bufs=2, space="PSUM") as ps_st,
            tc.tile_pool(name="ps_ot", bufs=2, space="PSUM") as ps_ot,
            tc.tile_pool(name="ps_aux", bufs=2, space="PSUM") as ps_aux,
        ):
            kT_sb = persist.tile([P, EO, NK], MDT, tag="kT")
            v_sb = persist.tile([P, NJT, HC * 65], MDT, tag="v")
            v_r = v_sb.rearrange("p n (h c) -> p n h c", c=65)
            ones_stage = onesp.tile([P, NJT, HC], f32, tag="ones_stage")
            nc.vector.memset(ones_stage[:], 1.0)
            nc.vector.tensor_copy(v_r[:, :, :, 64], ones_stage[:])
            # sel97[32s, s*64:(s+1)*64] = 1: lhsT selector so a K=97 matmul
            # broadcasts the reciprocal row at partition 32s across 64 output
            # partitions (engine access bases are limited to 0/32/64/96).
            sel_sb = onesp.tile([97, 4 * 64], f32, tag="sel")
            nc.vector.memset(sel_sb[:], 0.0)
            sel_r = sel_sb.rearrange("p (s c) -> p s c", c=64)
            for s in range(4):
                nc.vector.memset(sel_r[32 * s:32 * s + 1, s, :], 1.0)

            # ---------------- Phase K: kT and v projections ----------------
            qT_next = None
            wk_sb = weights.tile([P, DO, E], MDT, tag="w")
            nc.sync.dma_start(wk_sb[:], wkT_r[:])
            wv_sb = weights.tile([P, DO, E], MDT, tag="w")
            nc.sync.dma_start(wv_sb[:], wvT_r[:])
            wq_sb = weights.tile([P, DO, E], MDT, tag="w")
            nc.sync.dma_start(wq_sb[:], wqT_r[:])

            for jb in range(4):  # blocks of 512 j
                cblk = stream.tile([P, DO, 512], MDT, tag="blk")
                nc.sync.dma_start(cblk[:], cT_r[:, :, jb * 512:(jb + 1) * 512])
                # v[jtile, e] for the 4 j-tiles of this block, then kT e-tiles
                # in ascending order: attention head-pair hp needs only
                # kT[:, hp, :] and v, so hp0 can start before et 1..3 finish.
                for j2 in range(4):
                    jt = jb * 4 + j2
                    ps = ps_aux.tile([P, E], f32, tag="aux")
                    for do in range(DO):
                        nc.tensor.matmul(
                            ps[:], cblk[:, do, j2 * P:(j2 + 1) * P],
                            wv_sb[:, do, :],
                            start=(do == 0), stop=(do == DO - 1))
                    nc.vector.tensor_copy(
                        v_r[:, jt, :, 0:64],
                        ps.rearrange("p (h c) -> p h c", c=64))
                for et in range(EO):
                    ps = ps_aux.tile([P, 512], f32, tag="aux")
                    for do in range(DO):
                        nc.tensor.matmul(
                            ps[:], wk_sb[:, do, et * P:(et + 1) * P],
                            cblk[:, do, :],
                            start=(do == 0), stop=(do == DO - 1))
                    nc.vector.tensor_copy(kT_sb[:, et, jb * 512:(jb + 1) * 512],
                                          ps[:])

            wo_sb = weights.tile([P, EO, D], MDT, tag="w")
            nc.sync.dma_start(wo_sb[:], woT_r[:])

            # ---------------- Per i-block: Q proj, attention, out proj ------
            def qproj(ib):
                i0 = ib * IBW
                qT_ib = qpool.tile([P, EO, IBW], MDT, tag="q")
                xblk = stream.tile([P, DO, 512], MDT, tag="blk")
                nc.sync.dma_start(xblk[:], xT_r[:, :, i0:i0 + IBW])
                for et in range(EO):
                    ps = ps_aux.tile([P, 512], f32, tag="aux")
                    for do in range(DO):
                        nc.tensor.matmul(
                            ps[:], wq_sb[:, do, et * P:(et + 1) * P],
                            xblk[:, do, :],
                            start=(do == 0), stop=(do == DO - 1))
                    nc.vector.tensor_copy(qT_ib[:, et, :], ps[:])
                return qT_ib

            qT_next = qproj(0)
            for ib in range(NIB):
                i0 = ib * IBW
                qT_ib = qT_next

                oT_ib = opool.tile([P, EO, IBW], MDT, tag="o")
                # L rows parked at partitions {0,32,64,96} x free-slots {A,B};
                # unwritten partitions memset to 1.0 so the batched reciprocal
                # stays finite (they only ever multiply the selector's zeros).
                lrows = lrowsp.tile([97, 2, IBW], f32, tag="lr")
                nc.vector.memset(lrows[:], 1.0)

                for hp in range(4):  # head pairs (2hp, 2hp+1)
                    otA = ps_ot.tile([P, IBW], f32, tag="ot")
                    otB = ps_ot.tile([P, IBW], f32, tag="ot")
                    for jt in range(NJT):
                        st = ps_st.tile([P, 2, IBW], f32, tag="st")
                        # paired K=64 matmuls: head A rows 0-63, head B rows 64-127
                        for slot in (0, 1):
                            lo, hi = (0, 64) if slot == 0 else (64, 128)
                            nc.tensor.matmul(
                                st[:, slot, :],
                                kT_sb[lo:hi, hp, jt * P:(jt + 1) * P],
                                qT_ib[lo:hi, hp, :],
                                start=True, stop=True)
                        pt = ptpool.tile([P, 2, IBW], MDT, tag="pt")
                        nc.scalar.activation(
                            pt[:], st[:], mybir.ActivationFunctionType.Exp,
                            scale=SCALE)
                        for slot, ot in ((0, otA), (1, otB)):
                            hl = 2 * hp + slot
                            nc.tensor.matmul(
                                ot[0:65, :],
                                v_sb[:, jt, hl * 65:(hl + 1) * 65],
                                pt[:, slot, :],
                                start=(jt == 0), stop=(jt == NJT - 1))

                    # Release the OT psum banks quickly: copy the unnormalized
                    # O^T into its final SBUF slot and the L row aside; the
                    # reciprocal + normalization run batched after the hp loop
                    # so the PE keeps streaming the next head-pair's matmuls.
                    for head_off, ot in ((0, otA), (1, otB)):
                        base = 64 * head_off
                        nc.vector.tensor_copy(oT_ib[base:base + 64, hp, :],
                                              ot[0:64, :])
                        nc.vector.tensor_copy(
                            lrows[32 * hp:32 * hp + 1, head_off, :],
                            ot[64:65, :])

                # queue up the next i-block's Q projection so the PE has
                # work across the normalization boundary
                if ib + 1 < NIB:
                    qT_next = qproj(ib + 1)

                # batched softmax denominators for all 8 heads of this i-block
                rcrows = rcrowsp.tile([97, 2, IBW], f32, tag="rc")
                nc.vector.reciprocal(rcrows[:], lrows[:])
                for hl in range(HC):
                    hp, head_off = divmod(hl, 2)
                    base = 64 * head_off
                    bc = ps_aux.tile([P, IBW], f32, tag="aux")
                    nc.tensor.matmul(
                        bc[0:64, :], sel_sb[:, hp * 64:(hp + 1) * 64],
                        rcrows[:, head_off, :],
                        start=True, stop=True)
                    nc.vector.tensor_mul(
                        out=oT_ib[base:base + 64, hp, :],
                        in0=oT_ib[base:base + 64, hp, :],
                        in1=bc[0:64, :])

                # output projection for this i-block
                for it in range(4):
                    for ob in range(2):
                        ps = ps_aux.tile([P, IBW], f32, tag="aux")
                        for eo in range(EO):
                            nc.tensor.matmul(
                                ps[:],
                                oT_ib[:, eo, it * P:(it + 1) * P],
                                wo_sb[:, eo, ob * IBW:(ob + 1) * IBW],
                                start=(eo == 0), stop=(eo == EO - 1))
                        sg = stg.tile([P, IBW], f32, tag="sg")
                        nc.vector.tensor_copy(sg[:], ps[:])
                        r0 = i0 + it * P
                        nc.sync.dma_start(
                            out_ap[r0:r0 + P, ob * IBW:(ob + 1) * IBW], sg[:])

    nc.finalize()
    return nc


_NC_CACHE = None


def _get_nc():
    global _NC_CACHE
    if _NC_CACHE is None:
        _NC_CACHE = build_nc()
    return _NC_CACHE


def make_in_maps(x, context, Wq, Wk, Wv, Wo):
    in_maps = []
    for c in range(8):
        b, g = divmod(c, 2)
        es = slice(g * E, (g + 1) * E)
        in_maps.append({
            "xT": np.ascontiguousarray(x[b].T),
            "cT": np.ascontiguousarray(context[b].T),
            "wqT": np.ascontiguousarray(Wq[es].T),
            "wkT": np.ascontiguousarray(Wk[es].T),
            "wvT": np.ascontiguousarray(Wv[es].T),
            "woT": np.ascontiguousarray(Wo[:, es].T),
        })
    return in_maps


def kernel(**inputs):
    x = np.asarray(inputs["x"], dtype=np.float32)
    context = np.asarray(inputs["context"], dtype=np.float32)
    Wq = np.asarray(inputs["Wq"], dtype=np.float32)
    Wk = np.asarray(inputs["Wk"], dtype=np.float32)
    Wv = np.asarray(inputs["Wv"], dtype=np.float32)
    Wo = np.asarray(inputs["Wo"], dtype=np.float32)
    bo = np.asarray(inputs["bo"], dtype=np.float32)

    nc = _get_nc()
    in_maps = make_in_maps(x, context, Wq, Wk, Wv, Wo)
    res = run_bass_kernel_spmd(nc, in_maps, list(range(8)))
    out = np.zeros((B, NQ, D), np.float32)
    for c in range(8):
        out[c // 2] += res.results[c]["outp"]
    out += bo[None, None, :]
    return out


if __name__ == "__main__":
    nc = build_nc()
    print("built ok; instructions:", len(nc.inst_map))
